# revision 32
# baseline (speedup 1.0000x reference)
"""Trainium2 Bass kernel for GaborDownsampleBlock.

Computes: conv2d(x, gabor_filters(freq, theta, psi, sigma), stride=2, pad=1)
-> BatchNorm2d (training-mode batch stats) -> LeakyReLU(0.1).

Sharding: data-parallel over the batch dim (4 images per core on 8 cores).
Gabor/BN params are replicated. BN batch statistics are globalized with a
single 8-core AllGather of per-core partial sums.

Per-core layout: each input image is staged in SBUF zero-padded and split by
H-row parity across the 128 partitions — partitions 0-63 hold (i, even padded
rows), partitions 64-127 hold (i, odd padded rows) — then cast to bf16 on
GpSimd. Because KS=4/stride=2 the 4 kh taps pair two-per-parity, so the conv
is 8 full K=128 bf16 matmuls per PSUM tile (kw in 0..3, kh-pair in 0..1).
"""

import math

import numpy as np

import concourse.bacc as bacc
import concourse.mybir as mybir
import concourse.tile as tile
from concourse import bass_utils

N_CORES = 8
B, I, O, H, W = 32, 64, 128, 128, 128
B_LOC = B // N_CORES  # 4
OH = OW = 64
KS = 4
PI = 3.14  # module constant (not math.pi)
LIN = [-1.0, 0.0, 1.0, 2.0]  # linspace(-1, 2, 4)
HP = H // 2 + 1  # 65 padded-row slots per parity
WP = W + 2  # 130 padded cols
N_TILES = B_LOC * 8  # 32 psum tiles of [128, 512] per core
N_GLOBAL = float(B * OH * OW)  # BN sample count per channel

f32 = mybir.dt.float32
bf16 = mybir.dt.bfloat16
AF = mybir.ActivationFunctionType
ALU = mybir.AluOpType


def _gabor_weights(nc, cpool, thetaT, freqT, psiT, sigmaT):
    """Compute the 8 lhsT weight tiles as one [128, 8*O] bf16 buffer.

    Layout: partition (g, i) with g = kh parity, free (kw, pair, o);
    slice (kw*2+pair) holds w[o, i, kh=2*pair+g, kw].

    cos(f*rotx + psi) is evaluated via c = sin(pi/2 - a/4) followed by the
    exact quadruple-angle polynomial 8c^4 - 8c^2 + 1, which keeps every
    ScalarE Sin argument inside the LUT range [-pi, pi] without integer
    range-reduction. ACT functions are batched (one Square/Exp/Sin pass over
    [128, 1024]) so the activation table is loaded once per function.
    """
    th = cpool.tile([128, O], f32)
    nc.sync.dma_start(th[:], thetaT.ap())
    fr = cpool.tile([128, O], f32)
    nc.sync.dma_start(fr[:], freqT.ap())
    ps = cpool.tile([128, O], f32)
    nc.sync.dma_start(ps[:], psiT.ap())
    sg = cpool.tile([128, O], f32)
    nc.sync.dma_start(sg[:], sigmaT.ap())

    phv = cpool.tile([128, 1], f32)
    nc.gpsimd.memset(phv[:], math.pi / 2)
    # cos(t) = sin(pi/2 - t); theta in [0, 7pi/8] keeps the arg in range
    ct = cpool.tile([128, O], f32)
    nc.scalar.activation(ct[:], th[:], AF.Sin, bias=phv[:], scale=-1.0)
    st = cpool.tile([128, O], f32)
    nc.scalar.activation(st[:], th[:], AF.Sin)

    sp = cpool.tile([128, O], f32)
    nc.vector.tensor_scalar_add(sp[:], sg[:], 0.001)
    inv_s = cpool.tile([128, O], f32)
    nc.vector.reciprocal(inv_s[:], sp[:])
    c2 = cpool.tile([128, O], f32)
    nc.vector.tensor_mul(c2[:], inv_s[:], inv_s[:])
    nc.vector.tensor_scalar_mul(c2[:], c2[:], -0.5)

    s2 = cpool.tile([128, O], f32)
    nc.vector.tensor_mul(s2[:], sg[:], sg[:])
    rs2 = cpool.tile([128, O], f32)
    nc.vector.reciprocal(rs2[:], s2[:])
    nrm = cpool.tile([128, O], f32)
    nc.vector.tensor_scalar_mul(nrm[:], rs2[:], 1.0 / (2.0 * PI))

    # y = lin[kh] per-partition vectors; kh = 2*pair + (partition >= 64)
    ya = cpool.tile([128, 1], f32)
    nc.gpsimd.memset(ya[0:64, :], LIN[0])
    nc.gpsimd.memset(ya[64:128, :], LIN[1])
    yb = cpool.tile([128, 1], f32)
    nc.gpsimd.memset(yb[0:64, :], LIN[2])
    nc.gpsimd.memset(yb[64:128, :], LIN[3])

    # big scratch is aliased across phases to stay inside SBUF:
    #   rxb: rotx (live until the f*rotx pass)
    #   ryb: roty -> a = f*rotx+psi -> c = sin(pi/2-a/4) -> c^2
    #   sxb: rotx^2 -> rotx^2+roty^2 (in-place) -> exp(...) (in-place)
    #   wbuf: roty^2 -> poly/cos -> final f32 weights
    rxb = cpool.tile([128, 8 * O], f32)
    ryb = cpool.tile([128, 8 * O], f32)
    sxb = cpool.tile([128, 8 * O], f32)
    wbuf = cpool.tile([128, 8 * O], f32)
    wbufb = cpool.tile([128, 8 * O], bf16)
    ctx = cpool.tile([128, O], f32)
    stx = cpool.tile([128, O], f32)

    def sl(buf, k):
        return buf[:, k * O : (k + 1) * O]

    # rotx = x*cos + y*sin ; roty = y*cos - x*sin   (x=lin[kw], y=lin[kh])
    for kw in range(KS):
        nc.vector.tensor_scalar_mul(ctx[:], ct[:], LIN[kw])
        nc.vector.tensor_scalar_mul(stx[:], st[:], LIN[kw])
        for pair in range(2):
            yv = ya if pair == 0 else yb
            k = kw * 2 + pair
            nc.vector.scalar_tensor_tensor(
                sl(rxb, k), st[:], yv[:], ctx[:], op0=ALU.mult, op1=ALU.add
            )
            nc.vector.scalar_tensor_tensor(
                sl(ryb, k), ct[:], yv[:], stx[:], op0=ALU.mult, op1=ALU.subtract
            )
    nc.scalar.activation(sxb[:], rxb[:], AF.Square)
    nc.scalar.activation(wbuf[:], ryb[:], AF.Square)
    nc.vector.tensor_add(sxb[:], sxb[:], wbuf[:])  # rotx^2 + roty^2
    for k in range(8):
        nc.vector.tensor_mul(sl(sxb, k), sl(sxb, k), c2[:])
    nc.scalar.activation(sxb[:], sxb[:], AF.Exp)  # envelope, in-place

    for k in range(8):
        nc.vector.tensor_mul(sl(ryb, k), fr[:], sl(rxb, k))
        nc.vector.tensor_add(sl(ryb, k), sl(ryb, k), ps[:])
    # c = sin(pi/2 - a/4);  cos(a) = 8c^4 - 8c^2 + 1
    nc.scalar.activation(ryb[:], ryb[:], AF.Sin, bias=phv[:], scale=-0.25)
    nc.vector.tensor_mul(ryb[:], ryb[:], ryb[:])  # c^2
    nc.vector.tensor_scalar(
        wbuf[:], ryb[:], 1.0, -1.0, op0=ALU.mult, op1=ALU.add
    )  # c^2 - 1
    nc.vector.tensor_mul(wbuf[:], wbuf[:], ryb[:])  # c^2(c^2-1)
    nc.vector.tensor_scalar(
        wbuf[:], wbuf[:], 8.0, 1.0, op0=ALU.mult, op1=ALU.add
    )  # cos(a)
    nc.vector.tensor_mul(wbuf[:], wbuf[:], sxb[:])
    for k in range(8):
        nc.vector.tensor_mul(sl(wbuf, k), sl(wbuf, k), nrm[:])
    nc.vector.tensor_copy(wbufb[:], wbuf[:])
    return wbufb


def _body(nc, tc, xd, thetaT, freqT, psiT, sigmaT, gamd, betd, outd, groups,
          n_global=N_GLOBAL):
    with (
        tc.tile_pool(name="cpool", bufs=1) as cpool,
        tc.tile_pool(name="xpool", bufs=2) as xpool,
        tc.tile_pool(name="ppool", bufs=8, space="PSUM") as ppool,
        tc.tile_pool(name="rpool", bufs=1) as rpool,
        tc.tile_pool(name="spool", bufs=1) as spool,
        tc.tile_pool(name="dram", bufs=1, space="DRAM") as dram,
    ):
        wbufb = _gabor_weights(nc, cpool, thetaT, freqT, psiT, sigmaT)

        # ---------------- Conv + stats ----------------
        res = rpool.tile([128, N_TILES * 512], f32)
        sums = spool.tile([128, N_TILES], f32)
        sumsqs = spool.tile([128, N_TILES], f32)
        sqscr = spool.tile([128, 512], f32)

        xap = xd.ap()
        for b in range(B_LOC):
            # fp32 staging in padded parity layout, then bf16 cast on GpSimd
            xs = xpool.tile([128, HP * WP], f32, name="xs")
            xsv = xs.rearrange("p (s c) -> p s c", c=WP)
            nc.gpsimd.memset(xsv[0:64, 0, :], 0.0)
            nc.gpsimd.memset(xsv[64:128, HP - 1, :], 0.0)
            nc.gpsimd.memset(xsv[:, :, 0:1], 0.0)
            nc.gpsimd.memset(xsv[:, :, WP - 1 : WP], 0.0)
            # odd x rows -> G0 slots 1..64; even x rows -> G1 slots 0..63
            nc.sync.dma_start(xsv[0:64, 1:HP, 1 : W + 1], xap[b, :, 1::2, :])
            nc.sync.dma_start(xsv[64:128, 0 : HP - 1, 1 : W + 1],
                              xap[b, :, 0::2, :])
            xt = xpool.tile([128, HP * WP], bf16, name="xt")
            nc.gpsimd.tensor_copy(xt[:], xs[:])
            xv = xt.rearrange("p (s c) -> p s c", c=WP)

            for ohb in range(8):
                pt = ppool.tile([128, 512], f32, name="pt")
                k = 0
                for kw in range(KS):
                    for pair in range(2):
                        s0 = ohb * 8 + pair
                        rhs = xv[:, s0 : s0 + 8, kw : kw + 127 : 2]
                        lhsT = wbufb[:, (kw * 2 + pair) * O : (kw * 2 + pair + 1) * O]
                        nc.tensor.matmul(
                            pt[:], lhsT, rhs, start=(k == 0), stop=(k == 7)
                        )
                        k += 1
                t = b * 8 + ohb
                # PSUM -> resident copy + per-tile sum on DVE
                nc.vector.tensor_scalar(
                    res[:, t * 512 : (t + 1) * 512],
                    pt[:],
                    1.0,
                    0.0,
                    op0=ALU.mult,
                    op1=ALU.add,
                    accum_out=sums[:, t : t + 1],
                )
                # sum of squares on ACT (Square is its only conv-phase func)
                nc.scalar.activation(
                    sqscr[:], pt[:], AF.Square, accum_out=sumsqs[:, t : t + 1]
                )

        # ------- global BN stats (single 8-core AllGather + local sum) ------
        loc = spool.tile([128, 2], f32)
        nc.vector.reduce_sum(loc[:, 0:1], sums[:], axis=mybir.AxisListType.X)
        nc.vector.reduce_sum(loc[:, 1:2], sumsqs[:], axis=mybir.AxisListType.X)

        n_ranks = len(groups[0])
        bin_ = dram.tile([1, 256], f32)
        bout = dram.tile([n_ranks, 256], f32, addr_space="Shared")
        # dram[0, stat*128 + o] = loc[o, stat]
        nc.sync.dma_start(
            bin_[0:1, :].rearrange("a (s o) -> (a o) s", o=128), loc[:]
        )
        nc.gpsimd.collective_compute(
            "AllGather",
            ALU.bypass,
            replica_groups=groups,
            ins=[bin_.opt()],
            outs=[bout.opt()],
        )
        g = spool.tile([128, 2 * n_ranks], f32)
        gv = g.rearrange("o (s r) -> o s r", s=2)
        boutv = bout[:, :].rearrange("r (s o) -> o s r", o=128)
        for s in range(2):
            nc.sync.dma_start(gv[:, s, :], boutv[:, s, :])

        mn = spool.tile([128, 1], f32)
        nc.vector.reduce_sum(mn[:], gv[:, 0, :], axis=mybir.AxisListType.X)
        nc.vector.tensor_scalar_mul(mn[:], mn[:], 1.0 / n_global)
        ex2 = spool.tile([128, 1], f32)
        nc.vector.reduce_sum(ex2[:], gv[:, 1, :], axis=mybir.AxisListType.X)
        nc.vector.tensor_scalar_mul(ex2[:], ex2[:], 1.0 / n_global)
        var = spool.tile([128, 1], f32)
        nc.vector.tensor_mul(var[:], mn[:], mn[:])
        nc.vector.tensor_sub(var[:], ex2[:], var[:])
        nc.vector.tensor_scalar_add(var[:], var[:], 1e-5)
        rin = spool.tile([128, 1], f32)
        nc.vector.reciprocal(rin[:], var[:])
        inv = spool.tile([128, 1], f32)
        nc.scalar.activation(inv[:], rin[:], AF.Sqrt)

        gam = spool.tile([128, 1], f32)
        nc.sync.dma_start(gam[:], gamd.ap())
        bet = spool.tile([128, 1], f32)
        nc.sync.dma_start(bet[:], betd.ap())
        Asc = spool.tile([128, 1], f32)
        nc.vector.tensor_mul(Asc[:], gam[:], inv[:])
        Bsc = spool.tile([128, 1], f32)
        nc.vector.tensor_mul(Bsc[:], Asc[:], mn[:])
        nc.vector.tensor_sub(Bsc[:], bet[:], Bsc[:])

        # ---------------- normalize + LeakyReLU + store ----------------
        oap = outd.ap()
        for b in range(B_LOC):
            for ohb in range(8):
                t = b * 8 + ohb
                slc = res[:, t * 512 : (t + 1) * 512]
                # z = A*v + B, then leaky relu as max(0.1*z, z)
                nc.scalar.activation(
                    slc, slc, AF.Identity, bias=Bsc[:], scale=Asc[:]
                )
                nc.vector.scalar_tensor_tensor(
                    slc, slc, 0.1, slc, op0=ALU.mult, op1=ALU.max
                )
            nc.sync.dma_start(
                oap[b].rearrange("o h w -> o (h w)"),
                res[:, b * 8 * 512 : (b + 1) * 8 * 512],
            )


def build_nc(groups=None, n_global=N_GLOBAL):
    if groups is None:
        groups = [list(range(N_CORES))]
    nc = bacc.Bacc(
        "TRN2", target_bir_lowering=False, debug=False, num_devices=N_CORES
    )
    xd = nc.dram_tensor("x", [B_LOC, I, H, W], f32, kind="ExternalInput")
    thetaT = nc.dram_tensor("thetaT", [128, O], f32, kind="ExternalInput")
    freqT = nc.dram_tensor("freqT", [128, O], f32, kind="ExternalInput")
    psiT = nc.dram_tensor("psiT", [128, O], f32, kind="ExternalInput")
    sigmaT = nc.dram_tensor("sigmaT", [128, O], f32, kind="ExternalInput")
    gamd = nc.dram_tensor("gamma", [O, 1], f32, kind="ExternalInput")
    betd = nc.dram_tensor("beta", [O, 1], f32, kind="ExternalInput")
    outd = nc.dram_tensor("out", [B_LOC, O, OH, OW], f32, kind="ExternalOutput")
    with tile.TileContext(nc) as tc:
        _body(nc, tc, xd, thetaT, freqT, psiT, sigmaT, gamd, betd, outd,
              groups, n_global=n_global)
    nc.compile()
    return nc


_NC = None


def _install_ntff_hook():
    """Register the axon NTFF profiling hook if the image's antenv lacks it.

    ``run_bass_kernel_spmd(trace=True)`` under axon imports
    ``antenv.axon_hooks``; this container's antenv has no such module, but
    the ctypes hook implementation ships in ``trn_agent_boot``.
    """
    import sys
    import types

    try:
        import antenv.axon_hooks  # noqa: F401

        return
    except ImportError:
        pass
    try:
        import antenv
        from trn_agent_boot.trn_boot import _ntff_profile_via_ctypes

        hook = _ntff_profile_via_ctypes("/opt/axon/libaxon_pjrt.so")
        if hook is None:
            return
        mod = types.ModuleType("antenv.axon_hooks")
        state = {"hook": hook}
        mod.get_axon_ntff_profile_hook = lambda: state["hook"]
        mod.set_axon_ntff_profile_hook = lambda h: state.update(hook=h)
        sys.modules["antenv.axon_hooks"] = mod
        antenv.axon_hooks = mod
    except Exception:
        pass


def _marshal(x, freq, theta, psi, sigma, gamma, beta):
    """Build the 8 per-core input maps (host-side shard + replicate)."""

    def rep_t(p):
        pt = np.ascontiguousarray(p.T.astype(np.float32))  # [I, O]
        return np.concatenate([pt, pt], axis=0)  # [128, O]

    thetaT = rep_t(theta)
    freqT = rep_t(freq)
    psiT = rep_t(psi)
    sigmaT = rep_t(sigma)
    gam = np.ascontiguousarray(gamma.astype(np.float32).reshape(O, 1))
    bet = np.ascontiguousarray(beta.astype(np.float32).reshape(O, 1))
    in_maps = []
    for c in range(N_CORES):
        in_maps.append(
            {
                "x": np.ascontiguousarray(
                    x[c * B_LOC : (c + 1) * B_LOC].astype(np.float32)
                ),
                "thetaT": thetaT,
                "freqT": freqT,
                "psiT": psiT,
                "sigmaT": sigmaT,
                "gamma": gam,
                "beta": bet,
            }
        )
    return in_maps


def kernel(x, freq, theta, psi, sigma, gamma, beta, _trace=False):
    global _NC
    if _NC is None:
        _NC = build_nc()
    if _trace:
        _install_ntff_hook()
    in_maps = _marshal(x, freq, theta, psi, sigma, gamma, beta)
    res = bass_utils.run_bass_kernel_spmd(
        _NC, in_maps, core_ids=list(range(N_CORES)), trace=_trace
    )
    out = np.concatenate([res.results[c]["out"] for c in range(N_CORES)], axis=0)
    if _trace:
        kernel._last_results = res
    return out


# revision 34
# speedup vs baseline: 1.1211x; 1.1211x over previous
"""Trainium2 Bass kernel for GaborDownsampleBlock.

Computes: conv2d(x, gabor_filters(freq, theta, psi, sigma), stride=2, pad=1)
-> BatchNorm2d (training-mode batch stats) -> LeakyReLU(0.1).

Sharding: data-parallel over the batch dim (4 images per core on 8 cores).
Gabor/BN params are replicated. BN batch statistics are globalized with a
single 8-core AllGather of per-core partial sums.

Per-core layout: each input image is staged in SBUF zero-padded and split by
H-row parity across the 128 partitions — partitions 0-63 hold (i, even padded
rows), partitions 64-127 hold (i, odd padded rows) — then cast to bf16 on
GpSimd. Because KS=4/stride=2 the 4 kh taps pair two-per-parity, so the conv
is 8 full K=128 bf16 matmuls per PSUM tile (kw in 0..3, kh-pair in 0..1).
"""

import math

import numpy as np

import concourse.bacc as bacc
import concourse.mybir as mybir
import concourse.tile as tile
from concourse import bass_utils

N_CORES = 8
B, I, O, H, W = 32, 64, 128, 128, 128
B_LOC = B // N_CORES  # 4
OH = OW = 64
KS = 4
PI = 3.14  # module constant (not math.pi)
LIN = [-1.0, 0.0, 1.0, 2.0]  # linspace(-1, 2, 4)
HP = H // 2 + 1  # 65 padded-row slots per parity
WP = W + 2  # 130 padded cols
N_TILES = B_LOC * 8  # 32 psum tiles of [128, 512] per core
N_GLOBAL = float(B * OH * OW)  # BN sample count per channel

f32 = mybir.dt.float32
bf16 = mybir.dt.bfloat16
AF = mybir.ActivationFunctionType
ALU = mybir.AluOpType


def _gabor_weights(nc, cpool, thetaT, freqT, psiT, sigmaT):
    """Compute the 8 lhsT weight tiles as one [128, 8*O] bf16 buffer.

    Layout: partition (g, i) with g = kh parity, free (kw, pair, o);
    slice (kw*2+pair) holds w[o, i, kh=2*pair+g, kw].

    cos(f*rotx + psi) is evaluated via c = sin(pi/2 - a/4) followed by the
    exact quadruple-angle polynomial 8c^4 - 8c^2 + 1, which keeps every
    ScalarE Sin argument inside the LUT range [-pi, pi] without integer
    range-reduction. ACT functions are batched (one Square/Exp/Sin pass over
    [128, 1024]) so the activation table is loaded once per function.
    """
    th = cpool.tile([128, O], f32)
    nc.sync.dma_start(th[:], thetaT.ap())
    fr = cpool.tile([128, O], f32)
    nc.sync.dma_start(fr[:], freqT.ap())
    ps = cpool.tile([128, O], f32)
    nc.sync.dma_start(ps[:], psiT.ap())
    sg = cpool.tile([128, O], f32)
    nc.sync.dma_start(sg[:], sigmaT.ap())

    phv = cpool.tile([128, 1], f32)
    nc.gpsimd.memset(phv[:], math.pi / 2)
    # cos(t) = sin(pi/2 - t); theta in [0, 7pi/8] keeps the arg in range
    ct = cpool.tile([128, O], f32)
    nc.scalar.activation(ct[:], th[:], AF.Sin, bias=phv[:], scale=-1.0)
    st = cpool.tile([128, O], f32)
    nc.scalar.activation(st[:], th[:], AF.Sin)

    sp = cpool.tile([128, O], f32)
    nc.vector.tensor_scalar_add(sp[:], sg[:], 0.001)
    inv_s = cpool.tile([128, O], f32)
    nc.vector.reciprocal(inv_s[:], sp[:])
    c2 = cpool.tile([128, O], f32)
    nc.vector.tensor_mul(c2[:], inv_s[:], inv_s[:])
    nc.vector.tensor_scalar_mul(c2[:], c2[:], -0.5)

    s2 = cpool.tile([128, O], f32)
    nc.vector.tensor_mul(s2[:], sg[:], sg[:])
    rs2 = cpool.tile([128, O], f32)
    nc.vector.reciprocal(rs2[:], s2[:])
    nrm = cpool.tile([128, O], f32)
    nc.vector.tensor_scalar_mul(nrm[:], rs2[:], 1.0 / (2.0 * PI))

    # y = lin[kh] per-partition vectors; kh = 2*pair + (partition >= 64)
    ya = cpool.tile([128, 1], f32)
    nc.gpsimd.memset(ya[0:64, :], LIN[0])
    nc.gpsimd.memset(ya[64:128, :], LIN[1])
    yb = cpool.tile([128, 1], f32)
    nc.gpsimd.memset(yb[0:64, :], LIN[2])
    nc.gpsimd.memset(yb[64:128, :], LIN[3])

    # big scratch is aliased across phases to stay inside SBUF:
    #   rxb: rotx (live until the f*rotx pass)
    #   ryb: roty -> a = f*rotx+psi -> c = sin(pi/2-a/4) -> c^2
    #   sxb: rotx^2 -> rotx^2+roty^2 (in-place) -> exp(...) (in-place)
    #   wbuf: roty^2 -> poly/cos -> final f32 weights
    rxb = cpool.tile([128, 8 * O], f32)
    ryb = cpool.tile([128, 8 * O], f32)
    sxb = cpool.tile([128, 8 * O], f32)
    wbuf = cpool.tile([128, 8 * O], f32)
    wbufb = cpool.tile([128, 8 * O], bf16)
    ctx = cpool.tile([128, O], f32)
    stx = cpool.tile([128, O], f32)

    def sl(buf, k):
        return buf[:, k * O : (k + 1) * O]

    # rotx = x*cos + y*sin ; roty = y*cos - x*sin   (x=lin[kw], y=lin[kh])
    for kw in range(KS):
        nc.vector.tensor_scalar_mul(ctx[:], ct[:], LIN[kw])
        nc.vector.tensor_scalar_mul(stx[:], st[:], LIN[kw])
        for pair in range(2):
            yv = ya if pair == 0 else yb
            k = kw * 2 + pair
            nc.vector.scalar_tensor_tensor(
                sl(rxb, k), st[:], yv[:], ctx[:], op0=ALU.mult, op1=ALU.add
            )
            nc.vector.scalar_tensor_tensor(
                sl(ryb, k), ct[:], yv[:], stx[:], op0=ALU.mult, op1=ALU.subtract
            )
    nc.scalar.activation(sxb[:], rxb[:], AF.Square)
    nc.scalar.activation(wbuf[:], ryb[:], AF.Square)
    nc.vector.tensor_add(sxb[:], sxb[:], wbuf[:])  # rotx^2 + roty^2
    for k in range(8):
        nc.vector.tensor_mul(sl(sxb, k), sl(sxb, k), c2[:])
    nc.scalar.activation(sxb[:], sxb[:], AF.Exp)  # envelope, in-place

    for k in range(8):
        nc.vector.tensor_mul(sl(ryb, k), fr[:], sl(rxb, k))
        nc.vector.tensor_add(sl(ryb, k), sl(ryb, k), ps[:])
    # c = sin(pi/2 - a/4);  cos(a) = 8c^4 - 8c^2 + 1
    nc.scalar.activation(ryb[:], ryb[:], AF.Sin, bias=phv[:], scale=-0.25)
    nc.vector.tensor_mul(ryb[:], ryb[:], ryb[:])  # c^2
    nc.vector.tensor_scalar(
        wbuf[:], ryb[:], 1.0, -1.0, op0=ALU.mult, op1=ALU.add
    )  # c^2 - 1
    nc.vector.tensor_mul(wbuf[:], wbuf[:], ryb[:])  # c^2(c^2-1)
    nc.vector.tensor_scalar(
        wbuf[:], wbuf[:], 8.0, 1.0, op0=ALU.mult, op1=ALU.add
    )  # cos(a)
    nc.vector.tensor_mul(wbuf[:], wbuf[:], sxb[:])
    for k in range(8):
        nc.vector.tensor_mul(sl(wbuf, k), sl(wbuf, k), nrm[:])
    # f32 -> bf16 on ACT write path (CAST instructions are ~30us each here)
    nc.scalar.activation(wbufb[:], wbuf[:], AF.Copy)
    return wbufb


def _body(nc, tc, xd, thetaT, freqT, psiT, sigmaT, gamd, betd, outd, groups,
          n_global=N_GLOBAL):
    with (
        tc.tile_pool(name="cpool", bufs=1) as cpool,
        tc.tile_pool(name="xpool", bufs=2) as xpool,
        tc.tile_pool(name="ppool", bufs=8, space="PSUM") as ppool,
        tc.tile_pool(name="rpool", bufs=1) as rpool,
        tc.tile_pool(name="spool", bufs=1) as spool,
        tc.tile_pool(name="dram", bufs=1, space="DRAM") as dram,
    ):
        wbufb = _gabor_weights(nc, cpool, thetaT, freqT, psiT, sigmaT)

        # ---------------- Conv + stats ----------------
        res = rpool.tile([128, N_TILES * 512], f32)
        sums = spool.tile([128, N_TILES], f32)
        sumsqs = spool.tile([128, N_TILES], f32)
        sqscr = spool.tile([128, 512], f32)

        xap = xd.ap()
        for b in range(B_LOC):
            # fp32 staging in padded parity layout, then bf16 cast on GpSimd
            xs = xpool.tile([128, HP * WP], f32, name="xs")
            xsv = xs.rearrange("p (s c) -> p s c", c=WP)
            nc.gpsimd.memset(xsv[0:64, 0, :], 0.0)
            nc.gpsimd.memset(xsv[64:128, HP - 1, :], 0.0)
            nc.gpsimd.memset(xsv[:, :, 0:1], 0.0)
            nc.gpsimd.memset(xsv[:, :, WP - 1 : WP], 0.0)
            # odd x rows -> G0 slots 1..64; even x rows -> G1 slots 0..63
            nc.sync.dma_start(xsv[0:64, 1:HP, 1 : W + 1], xap[b, :, 1::2, :])
            nc.sync.dma_start(xsv[64:128, 0 : HP - 1, 1 : W + 1],
                              xap[b, :, 0::2, :])
            xt = xpool.tile([128, HP * WP], bf16, name="xt")
            nc.scalar.activation(xt[:], xs[:], AF.Copy)
            xv = xt.rearrange("p (s c) -> p s c", c=WP)

            for ohb in range(8):
                pt = ppool.tile([128, 512], f32, name="pt")
                k = 0
                for kw in range(KS):
                    for pair in range(2):
                        s0 = ohb * 8 + pair
                        rhs = xv[:, s0 : s0 + 8, kw : kw + 127 : 2]
                        lhsT = wbufb[:, (kw * 2 + pair) * O : (kw * 2 + pair + 1) * O]
                        nc.tensor.matmul(
                            pt[:], lhsT, rhs, start=(k == 0), stop=(k == 7)
                        )
                        k += 1
                t = b * 8 + ohb
                # PSUM -> resident copy + per-tile sum on DVE
                nc.vector.tensor_scalar(
                    res[:, t * 512 : (t + 1) * 512],
                    pt[:],
                    1.0,
                    0.0,
                    op0=ALU.mult,
                    op1=ALU.add,
                    accum_out=sums[:, t : t + 1],
                )
                # sum of squares on ACT (Square is its only conv-phase func)
                nc.scalar.activation(
                    sqscr[:], pt[:], AF.Square, accum_out=sumsqs[:, t : t + 1]
                )

        # ------- global BN stats (single 8-core AllGather + local sum) ------
        loc = spool.tile([128, 2], f32)
        nc.vector.reduce_sum(loc[:, 0:1], sums[:], axis=mybir.AxisListType.X)
        nc.vector.reduce_sum(loc[:, 1:2], sumsqs[:], axis=mybir.AxisListType.X)

        n_ranks = len(groups[0])
        bin_ = dram.tile([1, 256], f32)
        bout = dram.tile([n_ranks, 256], f32, addr_space="Shared")
        # dram[0, stat*128 + o] = loc[o, stat]
        nc.sync.dma_start(
            bin_[0:1, :].rearrange("a (s o) -> (a o) s", o=128), loc[:]
        )
        nc.gpsimd.collective_compute(
            "AllGather",
            ALU.bypass,
            replica_groups=groups,
            ins=[bin_.opt()],
            outs=[bout.opt()],
        )
        g = spool.tile([128, 2 * n_ranks], f32)
        gv = g.rearrange("o (s r) -> o s r", s=2)
        boutv = bout[:, :].rearrange("r (s o) -> o s r", o=128)
        for s in range(2):
            nc.sync.dma_start(gv[:, s, :], boutv[:, s, :])

        mn = spool.tile([128, 1], f32)
        nc.vector.reduce_sum(mn[:], gv[:, 0, :], axis=mybir.AxisListType.X)
        nc.vector.tensor_scalar_mul(mn[:], mn[:], 1.0 / n_global)
        ex2 = spool.tile([128, 1], f32)
        nc.vector.reduce_sum(ex2[:], gv[:, 1, :], axis=mybir.AxisListType.X)
        nc.vector.tensor_scalar_mul(ex2[:], ex2[:], 1.0 / n_global)
        var = spool.tile([128, 1], f32)
        nc.vector.tensor_mul(var[:], mn[:], mn[:])
        nc.vector.tensor_sub(var[:], ex2[:], var[:])
        nc.vector.tensor_scalar_add(var[:], var[:], 1e-5)
        rin = spool.tile([128, 1], f32)
        nc.vector.reciprocal(rin[:], var[:])
        inv = spool.tile([128, 1], f32)
        nc.scalar.activation(inv[:], rin[:], AF.Sqrt)

        gam = spool.tile([128, 1], f32)
        nc.sync.dma_start(gam[:], gamd.ap())
        bet = spool.tile([128, 1], f32)
        nc.sync.dma_start(bet[:], betd.ap())
        Asc = spool.tile([128, 1], f32)
        nc.vector.tensor_mul(Asc[:], gam[:], inv[:])
        Bsc = spool.tile([128, 1], f32)
        nc.vector.tensor_mul(Bsc[:], Asc[:], mn[:])
        nc.vector.tensor_sub(Bsc[:], bet[:], Bsc[:])

        # ---------------- normalize + LeakyReLU + store ----------------
        oap = outd.ap()
        for b in range(B_LOC):
            for ohb in range(8):
                t = b * 8 + ohb
                slc = res[:, t * 512 : (t + 1) * 512]
                # z = A*v + B, then leaky relu as max(0.1*z, z)
                nc.scalar.activation(
                    slc, slc, AF.Identity, bias=Bsc[:], scale=Asc[:]
                )
                nc.vector.scalar_tensor_tensor(
                    slc, slc, 0.1, slc, op0=ALU.mult, op1=ALU.max
                )
            nc.sync.dma_start(
                oap[b].rearrange("o h w -> o (h w)"),
                res[:, b * 8 * 512 : (b + 1) * 8 * 512],
            )


def build_nc(groups=None, n_global=N_GLOBAL):
    if groups is None:
        groups = [list(range(N_CORES))]
    nc = bacc.Bacc(
        "TRN2", target_bir_lowering=False, debug=False, num_devices=N_CORES
    )
    xd = nc.dram_tensor("x", [B_LOC, I, H, W], f32, kind="ExternalInput")
    thetaT = nc.dram_tensor("thetaT", [128, O], f32, kind="ExternalInput")
    freqT = nc.dram_tensor("freqT", [128, O], f32, kind="ExternalInput")
    psiT = nc.dram_tensor("psiT", [128, O], f32, kind="ExternalInput")
    sigmaT = nc.dram_tensor("sigmaT", [128, O], f32, kind="ExternalInput")
    gamd = nc.dram_tensor("gamma", [O, 1], f32, kind="ExternalInput")
    betd = nc.dram_tensor("beta", [O, 1], f32, kind="ExternalInput")
    outd = nc.dram_tensor("out", [B_LOC, O, OH, OW], f32, kind="ExternalOutput")
    with tile.TileContext(nc) as tc:
        _body(nc, tc, xd, thetaT, freqT, psiT, sigmaT, gamd, betd, outd,
              groups, n_global=n_global)
    nc.compile()
    return nc


_NC = None


def _install_ntff_hook():
    """Register the axon NTFF profiling hook if the image's antenv lacks it.

    ``run_bass_kernel_spmd(trace=True)`` under axon imports
    ``antenv.axon_hooks``; this container's antenv has no such module, but
    the ctypes hook implementation ships in ``trn_agent_boot``.
    """
    import sys
    import types

    try:
        import antenv.axon_hooks  # noqa: F401

        return
    except ImportError:
        pass
    try:
        import antenv
        from trn_agent_boot.trn_boot import _ntff_profile_via_ctypes

        hook = _ntff_profile_via_ctypes("/opt/axon/libaxon_pjrt.so")
        if hook is None:
            return
        mod = types.ModuleType("antenv.axon_hooks")
        state = {"hook": hook}
        mod.get_axon_ntff_profile_hook = lambda: state["hook"]
        mod.set_axon_ntff_profile_hook = lambda h: state.update(hook=h)
        sys.modules["antenv.axon_hooks"] = mod
        antenv.axon_hooks = mod
    except Exception:
        pass


def _marshal(x, freq, theta, psi, sigma, gamma, beta):
    """Build the 8 per-core input maps (host-side shard + replicate)."""

    def rep_t(p):
        pt = np.ascontiguousarray(p.T.astype(np.float32))  # [I, O]
        return np.concatenate([pt, pt], axis=0)  # [128, O]

    thetaT = rep_t(theta)
    freqT = rep_t(freq)
    psiT = rep_t(psi)
    sigmaT = rep_t(sigma)
    gam = np.ascontiguousarray(gamma.astype(np.float32).reshape(O, 1))
    bet = np.ascontiguousarray(beta.astype(np.float32).reshape(O, 1))
    in_maps = []
    for c in range(N_CORES):
        in_maps.append(
            {
                "x": np.ascontiguousarray(
                    x[c * B_LOC : (c + 1) * B_LOC].astype(np.float32)
                ),
                "thetaT": thetaT,
                "freqT": freqT,
                "psiT": psiT,
                "sigmaT": sigmaT,
                "gamma": gam,
                "beta": bet,
            }
        )
    return in_maps


def kernel(x, freq, theta, psi, sigma, gamma, beta, _trace=False):
    global _NC
    if _NC is None:
        _NC = build_nc()
    if _trace:
        _install_ntff_hook()
    in_maps = _marshal(x, freq, theta, psi, sigma, gamma, beta)
    res = bass_utils.run_bass_kernel_spmd(
        _NC, in_maps, core_ids=list(range(N_CORES)), trace=_trace
    )
    out = np.concatenate([res.results[c]["out"] for c in range(N_CORES)], axis=0)
    if _trace:
        kernel._last_results = res
    return out
